# revision 22
# baseline (speedup 1.0000x reference)
"""2-layer GCN on 8 Trainium2 NeuronCores via Bass/Tile.

Sharding: nodes row-sharded across the 8 cores (12500 each, padded to
12544); edges sharded by destination core and grouped by source shard
(the 8 GPSIMD groups).  Dense transforms run feats-on-partitions with
the weights stationary; the 16-dim xw is all-gathered into a per-core
SBUF table [128 partitions = 8 src shards x 16 feats, 12544 nodes];
messages are gathered with gpsimd.ap_gather, weighted and
segment-reduced on the vector engine ((dst,group) runs class-sorted by
length, host-baked region structure), permuted back to dst order with a
second ap_gather and combined across groups with one PE matmul
contracting the partition axis.  W2 is applied after the second spmm
(it commutes with the segment sum).
"""
import sys

for _p in ("/opt/trn_rl_repo",):
    if _p not in sys.path:
        sys.path.insert(0, _p)

import numpy as np
import ml_dtypes

N_CORES = 8
N_NODES = 100000
SHARD = 12500
SHARD_PAD = 12544          # 98*128
IN_DIM = 512
HID = 16
OUT = 7
CHUNK = 1024               # gather slots per ap_gather call (per group)
SLAB = 256                 # dense-phase node slab


def _preprocess(edge_src, edge_dst, edge_weight):
    core = (edge_dst // SHARD).astype(np.int64)
    grp = (edge_src // SHARD).astype(np.int64)
    dloc = (edge_dst - core * SHARD).astype(np.int64)
    sloc = (edge_src - grp * SHARD).astype(np.int32)

    key = (core * 8 + grp) * SHARD + dloc
    order = np.argsort(key, kind="stable")
    key_s = key[order]
    sloc_s = sloc[order]
    w_s = edge_weight[order].astype(np.float32)

    uk, start, cnt = np.unique(key_s, return_index=True, return_counts=True)
    run_core = (uk // (8 * SHARD)).astype(np.int64)
    run_grp = ((uk // SHARD) % 8).astype(np.int64)
    run_dst = (uk % SHARD).astype(np.int64)
    run_cnt = cnt.astype(np.int64)

    max_c = int(run_cnt.max())
    counts_cg = np.zeros((N_CORES, 8, max_c + 1), dtype=np.int64)
    np.add.at(counts_cg, (run_core, run_grp, run_cnt), 1)
    # spill-up capacities: runs may occupy a higher-class slot (zero-padded),
    # so per-class capacity only needs suffix_cap(c) >= max-stream suffix runs
    suf = counts_cg.reshape(64, max_c + 1)[:, ::-1].cumsum(axis=1)[:, ::-1]
    S = suf.max(axis=0)
    n_runs_per_class = np.zeros(max_c + 1, dtype=np.int64)
    acc_cap = 0
    for c in range(max_c, 0, -1):
        n_runs_per_class[c] = max(0, int(S[c]) - acc_cap)
        acc_cap += n_runs_per_class[c]
    n_runs_per_class[1] += 1   # shared zero-valued dummy run

    # slot layout: class regions, runs never straddle CHUNK boundaries
    regions = []               # (class, slot_start, run_start, n_runs)
    slot = 0
    run_base = 0
    for c in range(1, max_c + 1):
        n = int(n_runs_per_class[c])
        if n == 0:
            continue
        done = 0
        while done < n:
            room = CHUNK - (slot % CHUNK)
            fit = min(n - done, room // c)
            if fit == 0:
                slot += room
                continue
            regions.append((c, slot, run_base + done, fit))
            slot += fit * c
            done += fit
        run_base += n
    total_slots = ((slot + CHUNK - 1) // CHUNK) * CHUNK
    total_runs = run_base
    runs_pad = ((total_runs + 127) // 128) * 128

    class_run_off = np.zeros(max_c + 2, dtype=np.int64)
    for c in range(1, max_c + 1):
        class_run_off[c + 1] = class_run_off[c] + n_runs_per_class[c]
    run_slot = np.zeros(max(total_runs, 1), dtype=np.int64)
    for (c, s0, r0, n) in regions:
        run_slot[r0:r0 + n] = s0 + np.arange(n, dtype=np.int64) * c
    zero_run = class_run_off[2] - 1     # last class-1 run

    idx_all = np.zeros((N_CORES, 8, total_slots), dtype=np.int16)
    w_all = np.zeros((N_CORES, 8, total_slots), dtype=np.float32)
    perm_all = np.zeros((N_CORES, 8, SHARD_PAD), dtype=np.int16)

    for co in range(N_CORES):
        for g in range(8):
            sel = (run_core == co) & (run_grp == g)
            rc = run_cnt[sel]
            rd = run_dst[sel]
            rstart = start[sel]
            o = np.argsort(rc, kind="stable")
            rc, rd, rstart = rc[o], rd[o], rstart[o]
            # assign runs upward into class regions (spill-up, classes desc)
            free = n_runs_per_class.copy()
            free[1] -= 1                       # dummy zero run slot
            used = np.zeros(max_c + 1, dtype=np.int64)
            ridx = np.zeros(len(rc), dtype=np.int64)
            for c in range(max_c, 0, -1):
                m = rc == c
                k = int(m.sum())
                if k == 0:
                    continue
                idxs_of_runs = np.nonzero(m)[0]
                filled = 0
                cp = c
                while filled < k:
                    take = min(int(free[cp]), k - filled)
                    if take:
                        ridx[idxs_of_runs[filled:filled + take]] = (
                            class_run_off[cp] + used[cp] + np.arange(take))
                        used[cp] += take
                        free[cp] -= take
                        filled += take
                    cp += 1
                    assert cp <= max_c + 1 or filled == k
            slots = run_slot[ridx]
            ia = idx_all[co, g]
            wa = w_all[co, g]
            if len(rc):
                tot = int(rc.sum())
                within = np.arange(tot, dtype=np.int64) - np.repeat(
                    np.concatenate([[0], np.cumsum(rc)[:-1]]), rc)
                e_pos = np.repeat(rstart, rc) + within
                s_pos = np.repeat(slots, rc) + within
                ia[s_pos] = sloc_s[e_pos]
                wa[s_pos] = w_s[e_pos]
            pi = np.full(SHARD_PAD, zero_run, dtype=np.int64)
            pi[rd] = ridx
            perm_all[co, g, :] = pi.astype(np.int16)

    def wrap(a):   # [8, S] -> [128, S//16] (i -> partition 16g+i%16, slot i//16)
        S = a.shape[1]
        o = np.zeros((128, S // 16), dtype=a.dtype)
        for g in range(8):
            o[16 * g:16 * g + 16, :] = a[g].reshape(-1, 16).T
        return o

    idx_w = np.stack([wrap(idx_all[co]) for co in range(N_CORES)])
    perm_w = np.stack([wrap(perm_all[co]) for co in range(N_CORES)])
    w_exp = np.repeat(w_all, 16, axis=1)      # [cores, 128, S]

    return dict(idx=idx_w, perm=perm_w, w=w_exp, regions=regions,
                total_slots=total_slots, runs_pad=runs_pad)


def _build(total_slots, regions, runs_pad):
    from concourse import bass, bacc, tile, mybir
    dt = mybir.dt
    AF = mybir.ActivationFunctionType
    nc = bacc.Bacc("TRN2", target_bir_lowering=False, debug=False,
                   num_devices=N_CORES)

    fTd = nc.dram_tensor("fT", [IN_DIM, SHARD_PAD], dt.bfloat16, kind="ExternalInput").ap()
    mTd = nc.dram_tensor("mT", [IN_DIM, SHARD_PAD], dt.bfloat16, kind="ExternalInput").ap()
    m2d = nc.dram_tensor("m2T", [HID, SHARD_PAD], dt.float32, kind="ExternalInput").ap()
    W1d = nc.dram_tensor("W1", [IN_DIM, HID], dt.bfloat16, kind="ExternalInput").ap()
    b1d = nc.dram_tensor("b1", [HID, 1], dt.float32, kind="ExternalInput").ap()
    W2d = nc.dram_tensor("W2", [HID, OUT], dt.float32, kind="ExternalInput").ap()
    b2d = nc.dram_tensor("b2", [OUT, 1], dt.float32, kind="ExternalInput").ap()
    idxd = nc.dram_tensor("idx", [128, total_slots // 16], dt.int16, kind="ExternalInput").ap()
    permd = nc.dram_tensor("perm", [128, SHARD_PAD // 16], dt.int16, kind="ExternalInput").ap()
    wd = nc.dram_tensor("w", [128, total_slots], dt.bfloat16, kind="ExternalInput").ap()
    seld = nc.dram_tensor("sel", [128, HID], dt.float32, kind="ExternalInput").ap()
    outd = nc.dram_tensor("out", [OUT, SHARD_PAD], dt.float32, kind="ExternalOutput").ap()

    KB = IN_DIM // 128   # 4 k-blocks

    with tile.TileContext(nc) as tc:
        with tc.tile_pool(name="const", bufs=1) as cp, \
             tc.tile_pool(name="big", bufs=1) as bp, \
             tc.tile_pool(name="fp", bufs=3) as fp, \
             tc.tile_pool(name="wk", bufs=3) as wk, \
             tc.tile_pool(name="pf", bufs=6) as pf, \
             tc.tile_pool(name="ps", bufs=2, space="PSUM") as ps, \
             tc.tile_pool(name="dram", bufs=1, space="DRAM") as dp:

            W1 = cp.tile([128, KB, HID], dt.bfloat16)
            b1 = cp.tile([HID, 1], dt.float32)
            W2 = cp.tile([HID, OUT], dt.float32)
            b2 = cp.tile([OUT, 1], dt.float32)
            sel = cp.tile([128, HID], dt.float32)
            nc.sync.dma_start(out=W1[:, :, :], in_=W1d.rearrange("(a b) h -> b a h", b=128))
            nc.sync.dma_start(out=b1[:], in_=b1d[:])
            nc.sync.dma_start(out=W2[:], in_=W2d[:])
            nc.sync.dma_start(out=b2[:], in_=b2d[:])
            nc.sync.dma_start(out=sel[:], in_=seld[:])

            xT = bp.tile([HID, SHARD_PAD], dt.float32, tag="xT")
            table = bp.tile([128, SHARD_PAD], dt.float32, tag="table")
            st1 = bp.tile([128, runs_pad], dt.float32, tag="st1")

            # ---- dense layer 1: xT = W1^T (f.m) + b1 -------------------
            for off in range(0, SHARD_PAD, SLAB):
                f = fp.tile([128, KB, SLAB], dt.bfloat16, tag="f")
                m = fp.tile([128, KB, SLAB], dt.bfloat16, tag="m")
                nc.sync.dma_start(out=f[:, :, :], in_=fTd.rearrange("(a b) n -> b a n", b=128)[:, :, off:off + SLAB])
                nc.sync.dma_start(out=m[:, :, :], in_=mTd.rearrange("(a b) n -> b a n", b=128)[:, :, off:off + SLAB])
                nc.vector.tensor_tensor(out=f[:, :, :], in0=f[:, :, :], in1=m[:, :, :], op=mybir.AluOpType.mult)
                acc = ps.tile([HID, SLAB], dt.float32, tag="acc")
                for k in range(KB):
                    nc.tensor.matmul(out=acc[:, :], lhsT=W1[:, k, :], rhs=f[:, k, :],
                                     start=(k == 0), stop=(k == KB - 1))
                nc.scalar.activation(out=xT[:, off:off + SLAB], in_=acc[:, :],
                                     func=AF.Identity, bias=b1[:], scale=1.0)

            AGH = 9216     # first-half columns (36 dense slabs / 18 perm batches)

            def allgather(tag):
                for hi, (h0, hn) in enumerate(((0, AGH), (AGH, SHARD_PAD - AGH))):
                    gi = dp.tile([HID, hn], dt.float32, tag=f"gi{tag}_{hi}")
                    go = dp.tile([128, hn], dt.float32, tag=f"go{tag}_{hi}")
                    nc.sync.dma_start(out=gi[:], in_=xT[:, h0:h0 + hn])
                    nc.gpsimd.collective_compute(
                        "AllGather", mybir.AluOpType.bypass,
                        replica_groups=[list(range(N_CORES))],
                        ins=[gi.opt()], outs=[go.opt()])
                    nc.sync.dma_start(out=table[:, h0:h0 + hn], in_=go[:, :])

            def spmm(post):
                for cs in range(0, total_slots, CHUNK):
                    msg = wk.tile([128, CHUNK], dt.float32, tag="msg")
                    wch = pf.tile([128, CHUNK], dt.bfloat16, tag="wch")
                    ich = pf.tile([128, CHUNK // 16], dt.int16, tag="ich")
                    nc.sync.dma_start(out=wch[:, :], in_=wd[:, cs:cs + CHUNK])
                    nc.sync.dma_start(out=ich[:, :], in_=idxd[:, cs // 16:(cs + CHUNK) // 16])
                    nc.gpsimd.ap_gather(
                        out_ap=msg[:, :], in_ap=table[:, :],
                        idxs_ap=ich[:, :],
                        channels=128, num_elems=SHARD_PAD, d=1, num_idxs=CHUNK)
                    nc.vector.tensor_tensor(out=msg[:, :], in0=msg[:, :], in1=wch[:, :], op=mybir.AluOpType.mult)
                    for (c, s0, r0, n) in regions:
                        if s0 < cs or s0 >= cs + CHUNK:
                            continue
                        if c == 1:
                            nc.vector.tensor_copy(out=st1[:, r0:r0 + n], in_=msg[:, s0 - cs:s0 - cs + n])
                        else:
                            v = msg[:, s0 - cs:s0 - cs + n * c].rearrange("p (r c) -> p r c", c=c)
                            nc.vector.tensor_reduce(out=st1[:, r0:r0 + n], in_=v,
                                                    axis=mybir.AxisListType.X, op=mybir.AluOpType.add)
                for j in range(0, SHARD_PAD, 512):
                    jn = min(512, SHARD_PAD - j)
                    al = wk.tile([128, 512], dt.float32, tag="al")
                    pch = wk.tile([128, 32], dt.int16, tag="pch")
                    nc.sync.dma_start(out=pch[:, :jn // 16],
                                      in_=permd[:, j // 16:(j + jn) // 16])
                    nc.gpsimd.ap_gather(
                        out_ap=al[:, :jn], in_ap=st1[:, :],
                        idxs_ap=pch[:, :jn // 16],
                        channels=128, num_elems=runs_pad, d=1, num_idxs=jn)
                    acc = ps.tile([HID, 512], dt.float32, tag="acc2")
                    nc.tensor.matmul(out=acc[:, :jn], lhsT=sel[:, :], rhs=al[:, :jn],
                                     start=True, stop=True)
                    post(j, jn, acc)

            # ---- spmm 1: fused relu * mask2 into the post ------------
            def post1(j, jn, acc):
                m2 = wk.tile([HID, 512], dt.float32, tag="m2")
                nc.sync.dma_start(out=m2[:, :jn], in_=m2d[:, j:j + jn])
                nc.vector.scalar_tensor_tensor(
                    out=xT[:, j:j + jn], in0=acc[:, :jn], scalar=0.0,
                    in1=m2[:, :jn], op0=mybir.AluOpType.max,
                    op1=mybir.AluOpType.mult)

            allgather("1")
            spmm(post1)
            allgather("2")

            # ---- spmm 2: fused W2 + b2 + output DMA into the post ----
            def post2(j, jn, acc):
                h2c = wk.tile([HID, 512], dt.float32, tag="h2c")
                nc.scalar.activation(out=h2c[:, :jn], in_=acc[:, :jn],
                                     func=AF.Identity, scale=1.0)
                acc3 = ps.tile([OUT, 512], dt.float32, tag="acc3")
                nc.tensor.matmul(out=acc3[:, :jn], lhsT=W2[:, :], rhs=h2c[:, :jn],
                                 start=True, stop=True)
                o = wk.tile([OUT, 512], dt.float32, tag="o")
                nc.scalar.activation(out=o[:, :jn], in_=acc3[:, :jn],
                                     func=AF.Identity, bias=b2[:], scale=1.0)
                nc.sync.dma_start(out=outd[:, j:j + jn], in_=o[:, :jn])

            spmm(post2)

    nc.compile()
    return nc


def prepare(inputs):
    features = np.asarray(inputs["features"], dtype=np.float32)
    mask1 = np.asarray(inputs["mask1"], dtype=np.float32)
    mask2 = np.asarray(inputs["mask2"], dtype=np.float32)
    edge_src = np.asarray(inputs["edge_src"])
    edge_dst = np.asarray(inputs["edge_dst"])
    edge_weight = np.asarray(inputs["edge_weight"], dtype=np.float32)
    W1, b1, W2, b2 = (inputs["W1"], inputs["b1"], inputs["W2"], inputs["b2"])

    pp = _preprocess(edge_src, edge_dst, edge_weight)
    nc = _build(pp["total_slots"], pp["regions"], pp["runs_pad"])

    sel = np.zeros((128, HID), dtype=np.float32)
    for g in range(8):
        for f in range(HID):
            sel[16 * g + f, f] = 1.0

    in_maps = []
    for c in range(N_CORES):
        lo, hi = c * SHARD, (c + 1) * SHARD
        fT = np.zeros((IN_DIM, SHARD_PAD), dtype=np.float32)
        mT = np.zeros((IN_DIM, SHARD_PAD), dtype=np.float32)
        m2T = np.zeros((HID, SHARD_PAD), dtype=np.float32)
        fT[:, :SHARD] = features[lo:hi].T
        mT[:, :SHARD] = mask1[lo:hi].T
        m2T[:, :SHARD] = mask2[lo:hi].T
        fT = fT.astype(ml_dtypes.bfloat16)
        mT = mT.astype(ml_dtypes.bfloat16)
        in_maps.append({
            "fT": fT, "mT": mT, "m2T": m2T,
            "W1": np.asarray(W1, dtype=np.float32).reshape(IN_DIM, HID).astype(ml_dtypes.bfloat16),
            "b1": np.asarray(b1, dtype=np.float32).reshape(HID, 1),
            "W2": np.asarray(W2, dtype=np.float32).reshape(HID, OUT),
            "b2": np.asarray(b2, dtype=np.float32).reshape(OUT, 1),
            "idx": pp["idx"][c], "perm": pp["perm"][c],
            "w": pp["w"][c].astype(ml_dtypes.bfloat16),
            "sel": sel,
        })
    return nc, in_maps


def kernel(features, edge_src, edge_dst, edge_weight, mask1, mask2,
           W1, b1, W2, b2):
    from concourse.bass_utils import run_bass_kernel_spmd

    nc, in_maps = prepare(dict(
        features=features, edge_src=edge_src, edge_dst=edge_dst,
        edge_weight=edge_weight, mask1=mask1, mask2=mask2,
        W1=W1, b1=b1, W2=W2, b2=b2))

    res = run_bass_kernel_spmd(nc, in_maps, core_ids=list(range(N_CORES)))
    out = np.zeros((N_NODES, OUT), dtype=np.float32)
    for c in range(N_CORES):
        out[c * SHARD:(c + 1) * SHARD] = res.results[c]["out"][:, :SHARD].T
    return out

